# revision 4
# baseline (speedup 1.0000x reference)
"""GAT (graph attention) kernel for Trainium2, 8-core row-parallel SPMD.

Math (matches the reference exactly):
    h   = einsum('nm,hmf->hnf', x, W)                  # [H, N, F]
    ci  = h @ wi ; cj = h @ wj                         # [H, N]
    e   = exp(leaky_relu(ci[:,None] + cj[None,:], 0.2))
    adj = (graph > 0).T                                # mask[i, j] = graph[j, i] > 0
    att = softmax where adj, else 0
    y   = att @ h + x @ Wr + bias

Key algebraic factoring used on device (avoids any exp over the NxN matrix):
    exp(lrelu(t)) = max(exp(t), exp(0.2 t))            # lrelu slope 0.2
    with u=exp(ci), v=exp(cj), r=exp(-0.8 ci), rho=exp(-0.8 cj):
    e_ij = u_i * max(v_j, v_j * r_i * rho_j)
    u_i cancels between softmax numerator and denominator, so each core only
    materializes  Wt_ji = adj_ji * max(v_j, v'_j * r_i)   (v' = exp(0.2 cj))
    as an fp16 [j, i] tile (one 4x tensor_scalar + one 2x tensor_tensor pass),
    then aggregates on the PE:  psum[f, i] += H[j, f]^T @ Wt[j, i]
    with H carrying a ones-column so the softmax denominator falls out of the
    same matmul.

Sharding: core c owns output rows [c*1024, (c+1)*1024). Each core receives the
full x (to compute full h locally — no collectives), its column-slice of graph
(natural [j, i] layout, which is exactly the transposed mask the reference
uses), and its own row-slice of x for the residual + r_i.
"""

import numpy as np

import concourse.bass as bass
import concourse.tile as tile
from concourse import bacc, mybir
from concourse.bass_utils import run_bass_kernel_spmd
from concourse.masks import make_identity

N = 8192
IN_F = 256
HEADS = 4
HF = 64
OUT_F = HEADS * HF
NCORES = 8
ROWS = N // NCORES          # 1024 output rows per core
NJT = N // 128              # 64 j tiles of 128
MT = IN_F // 128            # 2 m tiles
ICH = ROWS // 512           # 2 moving-operand chunks of 512
HC = HF + 2                 # per-head columns in the h matmul: 64 h + ci + cj

F32 = mybir.dt.float32
F16 = mybir.dt.float16
I32 = mybir.dt.int32
ALU = mybir.AluOpType
AF = mybir.ActivationFunctionType


def _build_program():
    nc = bacc.Bacc("TRN2", target_bir_lowering=False, debug=False)

    x_d = nc.dram_tensor("x", [N, IN_F], F32, kind="ExternalInput")
    xr_d = nc.dram_tensor("xr", [ROWS, IN_F], F32, kind="ExternalInput")
    g_d = nc.dram_tensor("gcol", [N, ROWS], I32, kind="ExternalInput")
    w_d = nc.dram_tensor("weight", [HEADS, IN_F, HF], F32, kind="ExternalInput")
    wi_d = nc.dram_tensor("weight_i", [HEADS, HF, 1], F32, kind="ExternalInput")
    wj_d = nc.dram_tensor("weight_j", [HEADS, HF, 1], F32, kind="ExternalInput")
    wr_d = nc.dram_tensor("weight_r", [IN_F, OUT_F], F32, kind="ExternalInput")
    b_d = nc.dram_tensor("bias", [OUT_F], F32, kind="ExternalInput")
    y_d = nc.dram_tensor("y", [ROWS, OUT_F], F32, kind="ExternalOutput")

    with tile.TileContext(nc) as tc:
        _gat_body(tc, x_d, xr_d, g_d, w_d, wi_d, wj_d, wr_d, b_d, y_d)
    nc.compile()
    return nc


def _gat_body(tc, x_d, xr_d, g_d, w_d, wi_d, wj_d, wr_d, b_d, y_d):
    nc = tc.nc

    with tc.tile_pool(name="consts", bufs=1) as consts, \
         tc.tile_pool(name="persist", bufs=1) as persist:
        _gat_inner(tc, nc, consts, persist,
                   x_d, xr_d, g_d, w_d, wi_d, wj_d, wr_d, b_d, y_d)


def _gat_inner(tc, nc, consts, persist,
               x_d, xr_d, g_d, w_d, wi_d, wj_d, wr_d, b_d, y_d):
    ident = consts.tile([128, 128], F32, name="ident", tag="ident")
    make_identity(nc, ident)
    ones1 = consts.tile([1, 128], F32, name="ones1", tag="ones1")
    nc.gpsimd.memset(ones1[:], 1.0)

    # --- small weights ---
    wr_sb = [consts.tile([128, OUT_F], F32, name=f"wr{mt}", tag=f"wr{mt}") for mt in range(MT)]
    for mt in range(MT):
        nc.sync.dma_start(wr_sb[mt][:], wr_d[mt * 128:(mt + 1) * 128, :])
    bias_sb = consts.tile([1, OUT_F], F32, name="bias", tag="bias")
    nc.sync.dma_start(bias_sb[:], b_d.ap().rearrange("(a b) -> a b", a=1))
    wij_sb = []
    for h in range(HEADS):
        t = consts.tile([HF, 2], F32, name=f"wij{h}", tag=f"wij{h}")
        nc.sync.dma_start(t[:, 0:1], wi_d[h])
        nc.sync.dma_start(t[:, 1:2], wj_d[h])
        wij_sb.append(t)

    # RHSALL[mt]: per-head [W_h(64) | A_h | B_h] moving operand of the h matmul,
    # where A = W @ wi, B = W @ wj (so the same matmul emits h, ci, cj).
    rhsall = [consts.tile([128, HEADS * HC], F32, name=f"rhsall{mt}", tag=f"rhsall{mt}") for mt in range(MT)]
    for mt in range(MT):
        for h in range(HEADS):
            nc.sync.dma_start(rhsall[mt][:, h * HC:h * HC + HF],
                              w_d[h, mt * 128:(mt + 1) * 128, :])

    # --- transpose x (all rows) and xr (our rows) to [m, j] layout ---
    xT = [persist.tile([128, N], F32, name=f"xT{mt}", tag=f"xT{mt}") for mt in range(MT)]
    xrT = [persist.tile([128, ROWS], F32, name=f"xrT{mt}", tag=f"xrT{mt}") for mt in range(MT)]
    with tc.tile_pool(name="ph0", bufs=3) as ph0, \
         tc.tile_pool(name="ph0ps", bufs=4, space="PSUM") as ph0ps:
        for jt in range(NJT):
            xtile = ph0.tile([128, IN_F], F32, name="xload", tag="xload")
            nc.sync.dma_start(xtile[:], x_d[jt * 128:(jt + 1) * 128, :])
            for mt in range(MT):
                ps = ph0ps.tile([128, 128], F32, name="tps", tag="tps")
                nc.tensor.transpose(ps[:], xtile[:, mt * 128:(mt + 1) * 128], ident[:])
                eng = nc.vector if mt == 0 else nc.scalar
                if mt == 0:
                    nc.vector.tensor_copy(xT[mt][:, jt * 128:(jt + 1) * 128], ps[:])
                else:
                    nc.scalar.copy(xT[mt][:, jt * 128:(jt + 1) * 128], ps[:])
        for it in range(ROWS // 128):
            xrtile = ph0.tile([128, IN_F], F32, name="xrload", tag="xrload")
            nc.sync.dma_start(xrtile[:], xr_d[it * 128:(it + 1) * 128, :])
            for mt in range(MT):
                ps = ph0ps.tile([128, 128], F32, name="tps", tag="tps")
                nc.tensor.transpose(ps[:], xrtile[:, mt * 128:(mt + 1) * 128], ident[:])
                nc.vector.tensor_copy(xrT[mt][:, it * 128:(it + 1) * 128], ps[:])

        # W_h^T (for A/B columns): transpose the [m, f] weight slices.
        whT = [consts.tile([HF, IN_F], F32, name=f"whT{h}", tag=f"whT{h}") for h in range(HEADS)]
        for h in range(HEADS):
            for mt in range(MT):
                ps = ph0ps.tile([HF, 128], F32, name="wtps", tag="wtps", bufs=2)
                nc.tensor.transpose(ps[:], rhsall[mt][:, h * HC:h * HC + HF], ident[:])
                nc.vector.tensor_copy(whT[h][:, mt * 128:(mt + 1) * 128], ps[:])
        for h in range(HEADS):
            for mt in range(MT):
                psab = ph0ps.tile([128, 2], F32, name="abps", tag="abps", bufs=2)
                nc.tensor.matmul(psab[:], whT[h][:, mt * 128:(mt + 1) * 128],
                                 wij_sb[h][:], start=True, stop=True)
                nc.vector.tensor_copy(rhsall[mt][:, h * HC + HF:h * HC + HF + 2], psab[:])

    # --- h / ci / cj for all N rows ---
    H = [persist.tile([128, NJT, HF + 1], F16, name=f"H{h}", tag=f"H{h}") for h in range(HEADS)]
    cjT = persist.tile([128, NJT, HEADS], F32, name="cjT", tag="cjT")
    with tc.tile_pool(name="hps", bufs=3, space="PSUM") as hps:
        for jt in range(NJT):
            psh = hps.tile([128, HEADS * HC], F32, name="psh", tag="psh")
            for mt in range(MT):
                nc.tensor.matmul(psh[:], xT[mt][:, jt * 128:(jt + 1) * 128],
                                 rhsall[mt][:], start=(mt == 0), stop=(mt == MT - 1))
            psh_r = psh.rearrange("p (h c) -> p h c", h=HEADS)
            for h in range(HEADS):
                eng_copy = nc.scalar if h % 2 == 0 else nc.vector
                if h % 2 == 0:
                    nc.scalar.copy(H[h][:, jt, 0:HF], psh_r[:, h, 0:HF])
                else:
                    nc.vector.tensor_copy(H[h][:, jt, 0:HF], psh_r[:, h, 0:HF])
            nc.vector.tensor_copy(cjT[:, jt, :], psh_r[:, :, HF + 1])

    vs, vps = [], []
    for h in range(HEADS):
        # ones column for the softmax denominator
        nc.scalar.activation(H[h][:, :, HF], cjT[:, :, h], AF.Copy, bias=1.0, scale=0.0)
        v = persist.tile([128, NJT], F32, name=f"v{h}", tag=f"v{h}")
        nc.scalar.activation(v[:], cjT[:, :, h], AF.Exp)
        vp = persist.tile([128, NJT], F32, name=f"vp{h}", tag=f"vp{h}")
        nc.scalar.activation(vp[:], cjT[:, :, h], AF.Exp, scale=0.2)
        vs.append(v)
        vps.append(vp)

    # --- r_i = exp(-0.8 ci) for our rows, broadcast along partitions ---
    Rb = [persist.tile([128, ROWS], F16, name=f"Rb{h}", tag=f"Rb{h}") for h in range(HEADS)]
    with tc.tile_pool(name="cips", bufs=4, space="PSUM") as cips:
        for h in range(HEADS):
            rrow = persist.tile([1, ROWS], F16, name=f"rrow{h}", tag=f"rrow{h}")
            for ch in range(ICH):
                psci = cips.tile([2, 512], F32, name="psci", tag="psci")
                for mt in range(MT):
                    nc.tensor.matmul(psci[:], rhsall[mt][:, h * HC + HF:h * HC + HF + 2],
                                     xrT[mt][:, ch * 512:(ch + 1) * 512],
                                     start=(mt == 0), stop=(mt == MT - 1))
                nc.scalar.activation(rrow[0:1, ch * 512:(ch + 1) * 512],
                                     psci[0:1, :], AF.Exp, scale=-0.8)
            nc.gpsimd.partition_broadcast(Rb[h][:], rrow[:])

    # --- main loop: scores + aggregation ---
    with tc.tile_pool(name="psy", bufs=HEADS * ICH, space="PSUM") as psy_pool:
        psy = [[psy_pool.tile([HF + 1, 512], F32, name="psy", tag="psy") for _ in range(ICH)]
               for _ in range(HEADS)]
        with tc.tile_pool(name="mainl", bufs=3) as ml:
            for jb in range(NJT):
                g_int = ml.tile([128, ROWS], I32, name="gint", tag="gint")
                nc.sync.dma_start(g_int[:], g_d[jb * 128:(jb + 1) * 128, :])
                adj = ml.tile([128, ROWS], F16, name="adj", tag="adj")
                nc.gpsimd.tensor_scalar(adj[:], g_int[:], 0, None, ALU.is_gt)
                for h in range(HEADS):
                    mt_t = ml.tile([128, ROWS], F16, name="mt", tag="mt", bufs=4)
                    nc.vector.tensor_scalar(mt_t[:], Rb[h][:],
                                            vps[h][:, jb:jb + 1], vs[h][:, jb:jb + 1],
                                            ALU.mult, ALU.max)
                    wt_t = ml.tile([128, ROWS], F16, name="wt", tag="wt", bufs=4)
                    nc.vector.tensor_tensor(wt_t[:], mt_t[:], adj[:], ALU.mult)
                    for ch in range(ICH):
                        nc.tensor.matmul(psy[h][ch][:], H[h][:, jb, :],
                                         wt_t[:, ch * 512:(ch + 1) * 512],
                                         start=(jb == 0), stop=(jb == NJT - 1))

        # copy numerators/denominator out of PSUM (releases psy banks)
        ysb = [[persist.tile([HF + 1, 512], F32, name=f"ysb{h}_{ch}", tag=f"ysb{h}_{ch}") for ch in range(ICH)]
               for h in range(HEADS)]
        for h in range(HEADS):
            for ch in range(ICH):
                nc.scalar.copy(ysb[h][ch][:], psy[h][ch][:])

    # --- output: transpose to [i, f], divide by denominator, add residual ---
    with tc.tile_pool(name="outps", bufs=2, space="PSUM") as outps, \
         tc.tile_pool(name="outsb", bufs=2) as outsb:
        for it in range(ROWS // 128):
            ch, off = divmod(it * 128, 512)
            pso = outps.tile([128, HEADS, HF + 1], F32, name="pso", tag="pso")
            for h in range(HEADS):
                nc.tensor.transpose(pso[:, h, :], ysb[h][ch][:, off:off + 128],
                                    ident[0:HF + 1, 0:HF + 1])
            rden = outsb.tile([128, HEADS], F32, name="rden", tag="rden")
            nc.vector.reciprocal(rden[:], pso[:, :, HF])
            yatt = outsb.tile([128, OUT_F], F32, name="yatt", tag="yatt")
            for h in range(HEADS):
                nc.vector.tensor_scalar(yatt[:, h * HF:(h + 1) * HF], pso[:, h, 0:HF],
                                        rden[:, h:h + 1], None, ALU.mult)
            psr = outps.tile([128, OUT_F], F32, name="psr", tag="psr")
            for mt in range(MT):
                nc.tensor.matmul(psr[:], xrT[mt][:, it * 128:(it + 1) * 128],
                                 wr_sb[mt][:], start=(mt == 0), stop=False)
            nc.tensor.matmul(psr[:], ones1[:], bias_sb[:], start=False, stop=True)
            out_t = outsb.tile([128, OUT_F], F32, name="outt", tag="outt")
            nc.vector.tensor_tensor(out_t[:], yatt[:], psr[:], ALU.add)
            nc.sync.dma_start(y_d[it * 128:(it + 1) * 128, :], out_t[:])


_NC_CACHE = None


def _get_program():
    global _NC_CACHE
    if _NC_CACHE is None:
        _NC_CACHE = _build_program()
    return _NC_CACHE


def _make_in_maps(x, graph, weight, weight_i, weight_j, weight_r, bias):
    x = np.ascontiguousarray(x, dtype=np.float32)
    graph = np.ascontiguousarray(graph, dtype=np.int32)
    maps = []
    for c in range(NCORES):
        i0 = c * ROWS
        maps.append({
            "x": x,
            "xr": np.ascontiguousarray(x[i0:i0 + ROWS]),
            "gcol": np.ascontiguousarray(graph[:, i0:i0 + ROWS]),
            "weight": np.ascontiguousarray(weight, dtype=np.float32),
            "weight_i": np.ascontiguousarray(weight_i, dtype=np.float32),
            "weight_j": np.ascontiguousarray(weight_j, dtype=np.float32),
            "weight_r": np.ascontiguousarray(weight_r, dtype=np.float32),
            "bias": np.ascontiguousarray(bias, dtype=np.float32),
        })
    return maps


def _run(in_maps):
    nc = _get_program()
    res = run_bass_kernel_spmd(nc, in_maps, list(range(NCORES)))
    return np.concatenate([res.results[c]["y"] for c in range(NCORES)], axis=0)


def kernel(x, graph, weight, weight_i, weight_j, weight_r, bias):
    in_maps = _make_in_maps(x, graph, weight, weight_i, weight_j, weight_r, bias)
    return _run(in_maps).astype(np.float32)


# revision 36
# speedup vs baseline: 21063.5351x; 21063.5351x over previous
"""GAT (graph attention) kernel for Trainium2, 8-core row-parallel SPMD.

Math (matches the reference exactly):
    h   = einsum('nm,hmf->hnf', x, W)                  # [H, N, F]
    ci  = h @ wi ; cj = h @ wj                         # [H, N]
    e   = exp(leaky_relu(ci[:,None] + cj[None,:], 0.2))
    adj = (graph > 0).T                                # mask[i, j] = graph[j, i] > 0
    att = softmax where adj, else 0
    y   = att @ h + x @ Wr + bias

Key algebraic factoring used on device (avoids any exp over the NxN matrix):
    exp(lrelu(t)) = max(exp(t), exp(0.2 t))            # lrelu slope 0.2
    with u=exp(ci), v=exp(cj), r=exp(-0.8 ci), rho=exp(-0.8 cj):
    e_ij = u_i * max(v_j, v_j * r_i * rho_j)
    u_i cancels between softmax numerator and denominator, so each core only
    materializes  Wt_ji = adj_ji * max(v_j, v'_j * r_i)   (v' = exp(0.2 cj))
    in fp16 [j, i] layout: one dual-op tensor_scalar per head (mult+max with
    per-partition scalars) and ONE wide tensor_tensor over all 4 heads that
    multiplies by adj via a stride-0 (broadcast) access pattern. The PE then
    aggregates  psum[f, i] += H[j, f]^T @ Wt[j, i]  with H carrying a
    ones-column so the softmax denominator falls out of the same matmul.
    All elementwise work stays on the DVE: GPSIMD shares SBUF ports with the
    DVE, so offloading tensor ops there measured net-negative on hardware.

Sharding: core c owns output rows [c*1024, (c+1)*1024). Each core receives
x pre-transposed ([m, j] bf16, replicated - cheaper than an h all-gather),
its column-slice of graph (natural [j, i] layout, which is exactly the
transposed mask the reference uses), and its own row-slice of x.T (f32) for
the residual and r_i.
"""

import numpy as np

import concourse.bass as bass
import concourse.tile as tile
from concourse import bacc, mybir
from concourse.bass_utils import run_bass_kernel_spmd
from concourse.masks import make_identity

N = 8192
IN_F = 256
HEADS = 4
HF = 64
OUT_F = HEADS * HF
NCORES = 8
ROWS = N // NCORES          # 1024 output rows per core
NJT = N // 128              # 64 j tiles of 128
MT = IN_F // 128            # 2 m tiles
ICH = ROWS // 512           # 2 moving-operand chunks of 512
HC = HF + 2                 # per-head columns in the h matmul: 64 h + ci + cj

F32 = mybir.dt.float32
F16 = mybir.dt.float16
BF16 = mybir.dt.bfloat16
I32 = mybir.dt.int32
ALU = mybir.AluOpType
AF = mybir.ActivationFunctionType


def _build_program(loop_reps=None):
    nc = bacc.Bacc("TRN2", target_bir_lowering=False, debug=False)

    x_d = nc.dram_tensor("xt", [IN_F, N], BF16, kind="ExternalInput")
    xr_d = nc.dram_tensor("xrt", [IN_F, ROWS], F32, kind="ExternalInput")
    g_d = nc.dram_tensor("gcol", [N, ROWS], I32, kind="ExternalInput")
    w_d = nc.dram_tensor("weight", [HEADS, IN_F, HF], F32, kind="ExternalInput")
    wi_d = nc.dram_tensor("weight_i", [HEADS, HF, 1], F32, kind="ExternalInput")
    wj_d = nc.dram_tensor("weight_j", [HEADS, HF, 1], F32, kind="ExternalInput")
    wr_d = nc.dram_tensor("weight_r", [IN_F, OUT_F], F32, kind="ExternalInput")
    b_d = nc.dram_tensor("bias", [OUT_F], F32, kind="ExternalInput")
    y_d = nc.dram_tensor("y", [ROWS, OUT_F], F32, kind="ExternalOutput")

    with tile.TileContext(nc) as tc:
        if loop_reps is None:
            _gat_body(tc, x_d, xr_d, g_d, w_d, wi_d, wj_d, wr_d, b_d, y_d)
        else:
            with tc.For_i(0, loop_reps, 1):
                _gat_body(tc, x_d, xr_d, g_d, w_d, wi_d, wj_d, wr_d, b_d, y_d)
    nc.compile()
    return nc


def _gat_body(tc, x_d, xr_d, g_d, w_d, wi_d, wj_d, wr_d, b_d, y_d):
    nc = tc.nc

    with tc.tile_pool(name="consts", bufs=1) as consts, \
         tc.tile_pool(name="persist", bufs=1) as persist:
        _gat_inner(tc, nc, consts, persist,
                   x_d, xr_d, g_d, w_d, wi_d, wj_d, wr_d, b_d, y_d)


def _gat_inner(tc, nc, consts, persist,
               x_d, xr_d, g_d, w_d, wi_d, wj_d, wr_d, b_d, y_d):
    ident = consts.tile([128, 128], F32, name="ident", tag="ident")
    make_identity(nc, ident)
    ones1 = consts.tile([1, 128], F32, name="ones1", tag="ones1")
    nc.gpsimd.memset(ones1[:], 1.0)

    # --- x arrives pre-transposed (bf16) from the host; xr likewise (f32) ---
    xT = [persist.tile([128, N], BF16, name=f"xT{mt}", tag=f"xT{mt}") for mt in range(MT)]
    xrT = [persist.tile([128, ROWS], F32, name=f"xrT{mt}", tag=f"xrT{mt}") for mt in range(MT)]
    xrTb = [persist.tile([128, ROWS], BF16, name=f"xrTb{mt}", tag=f"xrTb{mt}") for mt in range(MT)]
    Hb = persist.tile([128, NJT, HEADS, HC], F16, name="Hb", tag="Hb")
    Rb = [persist.tile([128, ROWS], F16, name=f"Rb{h}", tag=f"Rb{h}") for h in range(HEADS)]
    cjT = persist.tile([128, NJT, HEADS], F32, name="cjT", tag="cjT")
    with tc.tile_pool(name="ph0", bufs=3) as ph0, \
         tc.tile_pool(name="ph0ps", bufs=3, space="PSUM") as ph0ps:
        # --- small weights ---
        wr_sb = [consts.tile([128, OUT_F], F32, name=f"wr{mt}", tag=f"wr{mt}") for mt in range(MT)]
        for mt in range(MT):
            nc.sync.dma_start(wr_sb[mt][:], wr_d[mt * 128:(mt + 1) * 128, :])
        bias_sb = consts.tile([1, OUT_F], F32, name="bias", tag="bias")
        nc.sync.dma_start(bias_sb[:], b_d.ap().rearrange("(a b) -> a b", a=1))
        wij_sb = []
        for h in range(HEADS):
            t = consts.tile([HF, 2], F32, name=f"wij{h}", tag=f"wij{h}")
            nc.sync.dma_start(t[:, 0:1], wi_d[h])
            nc.sync.dma_start(t[:, 1:2], wj_d[h])
            wij_sb.append(t)

        # RHSALL[mt]: per-head [W_h(64) | A_h | B_h] moving operand of the h matmul
        # (bf16, matching the bf16 xT stationary), where A = W @ wi, B = W @ wj
        # so the same matmul emits h, ci, cj.
        rhsall = [consts.tile([128, HEADS * HC], BF16, name=f"rhsall{mt}", tag=f"rhsall{mt}") for mt in range(MT)]
        wtmp = [consts.tile([128, HEADS * HF], F32, name=f"wtmp{mt}", tag=f"wtmp{mt}") for mt in range(MT)]
        for mt in range(MT):
            for h in range(HEADS):
                nc.sync.dma_start(wtmp[mt][:, h * HF:(h + 1) * HF],
                                  w_d[h, mt * 128:(mt + 1) * 128, :])
                nc.vector.tensor_copy(rhsall[mt][:, h * HC:h * HC + HF],
                                      wtmp[mt][:, h * HF:(h + 1) * HF])

        for mt in range(MT):
            for q in range(8):
                sl = slice(q * (N // 8), (q + 1) * (N // 8))
                eng = nc.sync if (mt * 8 + q) % 2 == 0 else nc.scalar
                eng.dma_start(xT[mt][:, sl], x_d[mt * 128:(mt + 1) * 128, sl])
            nc.sync.dma_start(xrT[mt][:], xr_d[mt * 128:(mt + 1) * 128, :])
            nc.vector.tensor_copy(xrTb[mt][:], xrT[mt][:])

        # W_h^T (for A/B columns): transpose the [m, f] weight slices.
        whT = [consts.tile([HF, IN_F], F32, name=f"whT{h}", tag=f"whT{h}") for h in range(HEADS)]
        for h in range(HEADS):
            for mt in range(MT):
                ps = ph0ps.tile([HF, 128], F32, name="wtps", tag="wtps", bufs=1)
                nc.tensor.transpose(ps[:], wtmp[mt][:, h * HF:(h + 1) * HF], ident[:])
                nc.vector.tensor_copy(whT[h][:, mt * 128:(mt + 1) * 128], ps[:])
        for h in range(HEADS):
            for mt in range(MT):
                psab = ph0ps.tile([128, 2], F32, name="abps", tag="abps", bufs=1)
                nc.tensor.matmul(psab[:], whT[h][:, mt * 128:(mt + 1) * 128],
                                 wij_sb[h][:], start=True, stop=True)
                nc.vector.tensor_copy(rhsall[mt][:, h * HC + HF:h * HC + HF + 2], psab[:])


        # --- r_i = exp(-0.8 ci) for our rows (early: unblocks main-loop lead) ---
        for h in range(HEADS):
            rrow = persist.tile([1, ROWS], F16, name=f"rrow{h}", tag=f"rrow{h}")
            for ch in range(ICH):
                psci = ph0ps.tile([2, 512], F32, name="psci", tag="psci", bufs=1)
                for mt in range(MT):
                    nc.tensor.matmul(psci[:], rhsall[mt][:, h * HC + HF:h * HC + HF + 2],
                                     xrTb[mt][:, ch * 512:(ch + 1) * 512],
                                     start=(mt == 0), stop=(mt == MT - 1))
                nc.scalar.activation(rrow[0:1, ch * 512:(ch + 1) * 512],
                                     psci[0:1, :], AF.Exp, scale=-0.8)
            nc.gpsimd.partition_broadcast(Rb[h][:], rrow[:])

        # --- h / ci / cj for all N rows (same psum pool: pipelines with x loads) ---
        for jt in range(NJT):
            psh = ph0ps.tile([128, HEADS * HC], F32, name="psh", tag="psh", bufs=5)
            for mt in range(MT):
                nc.tensor.matmul(psh[:], xT[mt][:, jt * 128:(jt + 1) * 128],
                                 rhsall[mt][:], start=(mt == 0), stop=(mt == MT - 1))
            psh_r = psh.rearrange("p (h c) -> p h c", h=HEADS)
            hdst = Hb[:, jt].rearrange("p a b -> p (a b)")
            if jt % 2 == 0:
                nc.scalar.copy(hdst, psh[:])
            else:
                nc.vector.tensor_copy(hdst, psh[:])
            nc.scalar.copy(cjT[:, jt, :], psh_r[:, :, HF + 1])

    vs, vps = [], []
    for h in range(HEADS):
        # ones column for the softmax denominator (overwrites the unused ci slot)
        nc.scalar.activation(Hb[:, :, h, HF], cjT[:, :, h], AF.Copy, bias=1.0, scale=0.0)
        v = persist.tile([128, NJT], F32, name=f"v{h}", tag=f"v{h}")
        vp = persist.tile([128, NJT], F32, name=f"vp{h}", tag=f"vp{h}")
        for q in range(4):
            sl = slice(q * 16, (q + 1) * 16)
            nc.scalar.activation(v[:, sl], cjT[:, sl, h], AF.Exp)
            nc.scalar.activation(vp[:, sl], cjT[:, sl, h], AF.Exp, scale=0.2)
        vs.append(v)
        vps.append(vp)

    # --- main loop: scores + aggregation ---
    with tc.tile_pool(name="psy", bufs=HEADS * ICH, space="PSUM") as psy_pool:
        psy = [[psy_pool.tile([HF + 1, 512], F32, name="psy", tag="psy") for _ in range(ICH)]
               for _ in range(HEADS)]
        with tc.tile_pool(name="mainl", bufs=3) as ml:
            for jb in range(NJT):
                g_int = ml.tile([128, ROWS], I32, name="gint", tag="gint", bufs=3)
                nc.sync.dma_start(g_int[:], g_d[jb * 128:(jb + 1) * 128, :])
                adj = ml.tile([128, ROWS], F16, name="adj", tag="adj", bufs=3)
                nc.scalar.activation(adj[:], g_int[:], AF.Sign)
                # all 4 heads' scores in one tile; mask applied by a single wide
                # tensor_tensor with a stride-0 (broadcast) read of adj
                mt4 = ml.tile([128, HEADS, ROWS], F16, name="mt4", tag="mt4", bufs=3)
                for h in range(HEADS):
                    nc.vector.tensor_scalar(mt4[:, h, :], Rb[h][:],
                                            vps[h][:, jb:jb + 1], vs[h][:, jb:jb + 1],
                                            ALU.mult, ALU.max)
                wt4 = ml.tile([128, HEADS, ROWS], F16, name="wt4", tag="wt4", bufs=3)
                adj1 = adj.rearrange("p (o f) -> p o f", o=1)
                adj4 = bass.AP(tensor=adj1.tensor, offset=adj1.offset,
                               ap=[adj1.ap[0], [0, HEADS], adj1.ap[2]])
                nc.vector.tensor_tensor(wt4[:], mt4[:], adj4[:], ALU.mult)
                for h in range(HEADS):
                    for ch in range(ICH):
                        nc.tensor.matmul(psy[h][ch][:], Hb[:, jb, h, 0:HF + 1],
                                         wt4[:, h, ch * 512:(ch + 1) * 512],
                                         start=(jb == 0), stop=(jb == NJT - 1))

        # copy numerators/denominator out of PSUM (releases psy banks)
        ysb = [[persist.tile([HF + 1, 512], F32, name=f"ysb{h}_{ch}", tag=f"ysb{h}_{ch}") for ch in range(ICH)]
               for h in range(HEADS)]
        for h in range(HEADS):
            for ch in range(ICH):
                if (h + ch) % 2 == 0:
                    nc.scalar.copy(ysb[h][ch][:], psy[h][ch][:])
                else:
                    nc.vector.tensor_copy(ysb[h][ch][:], psy[h][ch][:])

    # --- output: transpose to [i, f], divide by denominator, add residual ---
    with tc.tile_pool(name="outps", bufs=3, space="PSUM") as outps, \
         tc.tile_pool(name="outsb", bufs=3) as outsb:
        for it in range(ROWS // 128):
            ch, off = divmod(it * 128, 512)
            pso = outps.tile([128, HEADS, HF + 1], F32, name="pso", tag="pso")
            for h in range(HEADS):
                nc.tensor.transpose(pso[:, h, :], ysb[h][ch][:, off:off + 128],
                                    ident[0:HF + 1, 0:HF + 1])
            rden = outsb.tile([128, HEADS], F32, name="rden", tag="rden")
            nc.vector.reciprocal(rden[:], pso[:, :, HF])
            yatt = outsb.tile([128, OUT_F], F32, name="yatt", tag="yatt")
            for h in range(HEADS):
                nc.scalar.activation(yatt[:, h * HF:(h + 1) * HF], pso[:, h, 0:HF],
                                     AF.Copy, scale=rden[:, h:h + 1])
            psr = outps.tile([128, OUT_F], F32, name="psr", tag="psr")
            for mt in range(MT):
                nc.tensor.matmul(psr[:], xrT[mt][:, it * 128:(it + 1) * 128],
                                 wr_sb[mt][:], start=(mt == 0), stop=False)
            nc.tensor.matmul(psr[:], ones1[:], bias_sb[:], start=False, stop=True)
            out_t = outsb.tile([128, OUT_F], F32, name="outt", tag="outt")
            nc.vector.tensor_tensor(out_t[:], yatt[:], psr[:], ALU.add)
            nc.sync.dma_start(y_d[it * 128:(it + 1) * 128, :], out_t[:])


_NC_CACHE = {}


def _get_program(loop_reps=None):
    if loop_reps not in _NC_CACHE:
        _NC_CACHE[loop_reps] = _build_program(loop_reps)
    return _NC_CACHE[loop_reps]


def _make_in_maps(x, graph, weight, weight_i, weight_j, weight_r, bias):
    import ml_dtypes
    x = np.asarray(x, dtype=np.float32)
    graph = np.ascontiguousarray(graph, dtype=np.int32)
    xt = np.ascontiguousarray(x.T)                      # [IN_F, N] f32
    xt_bf = xt.astype(ml_dtypes.bfloat16)               # replicated operand, bf16
    maps = []
    for c in range(NCORES):
        i0 = c * ROWS
        maps.append({
            "xt": xt_bf,
            "xrt": np.ascontiguousarray(xt[:, i0:i0 + ROWS]),
            "gcol": np.ascontiguousarray(graph[:, i0:i0 + ROWS]),
            "weight": np.ascontiguousarray(weight, dtype=np.float32),
            "weight_i": np.ascontiguousarray(weight_i, dtype=np.float32),
            "weight_j": np.ascontiguousarray(weight_j, dtype=np.float32),
            "weight_r": np.ascontiguousarray(weight_r, dtype=np.float32),
            "bias": np.ascontiguousarray(bias, dtype=np.float32),
        })
    return maps


def _run(in_maps, loop_reps=None):
    nc = _get_program(loop_reps)
    res = run_bass_kernel_spmd(nc, in_maps, list(range(NCORES)))
    return np.concatenate([res.results[c]["y"] for c in range(NCORES)], axis=0)


def kernel(x, graph, weight, weight_i, weight_j, weight_r, bias):
    in_maps = _make_in_maps(x, graph, weight, weight_i, weight_j, weight_r, bias)
    return _run(in_maps).astype(np.float32)

